# revision 36
# baseline (speedup 1.0000x reference)
"""BertLSTMCrf TRN2 kernel: 8-core Bass/Tile implementation (v2).

Sharding: 8 cores = 2 LSTM directions x 4 time-chunks of 128 steps, each
with a 32-step zero-state warmup.  v2 changes vs baseline:
  - bert band transposed + fp16 hi/lo split on HOST -> no PE transposes,
    no on-device casts; x arrives as [128, DC, SPAN, B] fp16 hi/lo.
  - 2-pass projection (w_hi*x_hi + w_hi*x_lo): x is exact in hi+lo,
    weights fp16-rounded (1-pass in the 32-step warmup region).
  - emissions fused into the recurrence: per-step [B,T] matmul with h16
    stationary (b-cores store time-reversed via partition-id addressing),
    so phase B + the h HBM round-trip are gone; b_eff folded into
    trans/vinit on host.
  - viterbi: mask folded into the precomputed aspan (masked steps become
    identity transitions) -> no copy_predicated in the sequential loops.
  - LSTM gate order [i,f,o,g] per chunk -> one fused sigmoid; c-update
    muls/adds on gpsimd to unload DVE.
"""
import numpy as np
from contextlib import ExitStack

import concourse.bass as bass
import concourse.mybir as mybir
import concourse.tile as tile
from concourse import bacc
from concourse.bass_utils import run_bass_kernel_spmd

F32 = mybir.dt.float32
F16 = mybir.dt.float16
I32 = mybir.dt.int32
U8 = mybir.dt.uint8
AF = mybir.ActivationFunctionType
ALU = mybir.AluOpType

B, S, D, H, T = 64, 512, 768, 384, 9
NCH, CH, W = 4, 128, 16
WIN = 8
SPAN = CH + W                    # 160
NW = SPAN // WIN                 # 20
NWARM = W // WIN                 # 4
PASSES = 1                       # 1: hi*hi   2: + hi(w)*lo(x)   3: + lo(w)*hi(x)
VCH, VW = 64, 32
SPAN_V = VW + VCH + VW           # 192
PADW = VW + S + VW               # 640
DC, HC, GC = D // 128, H // 128, 4 * H // 128

_cache = {}


def _bc(ap, n, pos):
    """Insert a broadcast (step-0) free dim of length n at free position."""
    a = [list(x) for x in ap.ap]
    a.insert(1 + pos, [0, n])
    return bass.AP(tensor=ap.tensor, offset=ap.offset, ap=a)


def _bccol(ap, n):
    """Turn a [P,1] column AP into a [P,n] broadcast AP."""
    a = [list(x) for x in ap.ap]
    assert a[-1][1] == 1
    a[-1] = [0, n]
    return bass.AP(tensor=ap.tensor, offset=ap.offset, ap=a)


def _build():
    nc = bacc.Bacc("TRN2", target_bir_lowering=False, debug=False,
                   num_devices=8)
    dt = nc.dram_tensor
    xt_hi = dt("xt_hi", [128, DC, SPAN, B], F16, kind="ExternalInput").ap()
    xt_lo = dt("xt_lo", [128, DC, SPAN, B], F16, kind="ExternalInput").ap()
    wih_hi = dt("wih_hi", [128, DC, 4 * H], F16, kind="ExternalInput").ap()
    if PASSES >= 3:
        wih_lo = dt("wih_lo", [128, DC, 4 * H], F16,
                    kind="ExternalInput").ap()
    whh16 = dt("whh16", [128, HC, 4 * H], F16, kind="ExternalInput").ap()
    bias_w = dt("bias_w", [128, GC], F32, kind="ExternalInput").ap()
    bias_m = dt("bias_m", [128, GC], F32, kind="ExternalInput").ap()
    weff16 = dt("weff16", [128, HC, T], F16, kind="ExternalInput").ap()
    transb = dt("transb", [T, T], F32, kind="ExternalInput").ap()
    diagm = dt("diagm", [T, T], F32, kind="ExternalInput").ap()
    iota81 = dt("iota81", [T, T], F32, kind="ExternalInput").ap()
    iota9 = dt("iota9", [T], F32, kind="ExternalInput").ap()
    end9d = dt("end9d", [B, T], F32, kind="ExternalInput").ap()
    vinit = dt("vinit", [B, T], F32, kind="ExternalInput").ap()
    vinitsel = dt("vinitsel", [B, T], F32, kind="ExternalInput").ap()
    masknot = dt("masknot", [B, PADW], U8, kind="ExternalInput").ap()

    emT_bounce = dt("emT_bounce", [B, CH, T], F16)
    emg_bounce = dt("emg_bounce", [8, B, CH, T], F16)

    tags_out = dt("tags", [B, VCH], I32, kind="ExternalOutput").ap()

    with tile.TileContext(nc) as tc, ExitStack() as ctx:
        cpool = ctx.enter_context(tc.tile_pool(name="consts", bufs=1))
        wih_hi_sb = cpool.tile([128, DC, 4 * H], F16)
        nc.sync.dma_start(wih_hi_sb[:], wih_hi)
        if PASSES >= 3:
            wih_lo_sb = cpool.tile([128, DC, 4 * H], F16)
            nc.sync.dma_start(wih_lo_sb[:], wih_lo)
        whh_sb = cpool.tile([128, HC, 4 * H], F16)
        nc.sync.dma_start(whh_sb[:], whh16)
        bias_w_sb = cpool.tile([128, GC], F32)
        bias_m_sb = cpool.tile([128, GC], F32)
        nc.sync.dma_start(bias_w_sb[:], bias_w)
        nc.sync.dma_start(bias_m_sb[:], bias_m)
        weff_sb = cpool.tile([128, HC, T], F16)
        nc.sync.dma_start(weff_sb[:], weff16)

        def dma_bcast(dst, src):
            p = dst.shape[0]
            src_b = bass.AP(tensor=src.tensor, offset=src.offset,
                            ap=[[0, p]] + [list(x) for x in src.ap])
            nc.sync.dma_start(dst[:], src_b)

        transb_sb = cpool.tile([B, T, T], F32)
        dma_bcast(transb_sb, transb)
        diagm_sb = cpool.tile([B, T, T], F32)
        dma_bcast(diagm_sb, diagm)
        iota81_sb = cpool.tile([B, T, T], F32)
        dma_bcast(iota81_sb, iota81)
        iota9_sb = cpool.tile([B, T], F32)
        dma_bcast(iota9_sb, iota9)
        end9_sb = cpool.tile([B, T], F32)
        nc.sync.dma_start(end9_sb[:], end9d)
        vinit_sb = cpool.tile([B, T], F32)
        nc.sync.dma_start(vinit_sb[:], vinit)
        vinitsel_sb = cpool.tile([B, T], F32)
        nc.sync.dma_start(vinitsel_sb[:], vinitsel)

        # b-cores (partition_id >= 4) store emissions time-reversed so the
        # allgathered chunks line up forward; em index = t0 + pb*(CH-1-2*t0)
        pb = nc.partition_id() // 4

        # ---------- phase A: projection + recurrence + fused emission ----
        with tc.tile_pool(name="xt", bufs=2) as xtp, \
             tc.tile_pool(name="xg", bufs=2) as xgp, \
             tc.tile_pool(name="st", bufs=1) as stp, \
             tc.tile_pool(name="gt", bufs=4) as gtp, \
             tc.tile_pool(name="h16p", bufs=2) as h16p, \
             tc.tile_pool(name="emsb", bufs=1) as emp, \
             tc.tile_pool(name="xgps", bufs=3, space="PSUM") as xgps, \
             tc.tile_pool(name="gps", bufs=1, space="PSUM") as gps, \
             tc.tile_pool(name="emps", bufs=1, space="PSUM") as emps:

            c32 = stp.tile([128, HC, B], F32)
            nc.vector.memset(c32[:], 0.0)
            h16 = h16p.tile([128, HC, B], F16, tag="h16")
            nc.vector.memset(h16[:], 0.0)
            emT_sb = emp.tile([B, CH, T], F16)

            def dma_window(w):
                warm = w < NWARM
                xh = xtp.tile([128, DC, WIN, B], F16, tag="xh")
                nc.sync.dma_start(xh[:],
                                  xt_hi[:, :, w * WIN:(w + 1) * WIN, :])
                xl = None
                if not warm and PASSES >= 2:
                    xl = xtp.tile([128, DC, WIN, B], F16, tag="xl")
                    nc.sync.dma_start(xl[:],
                                      xt_lo[:, :, w * WIN:(w + 1) * WIN, :])
                return xh, xl

            def proj_mm(w, xh, xl, m):
                warm = w < NWARM
                passes = [(wih_hi_sb, xh)]
                if not warm:
                    if PASSES >= 2:
                        passes.append((wih_hi_sb, xl))
                    if PASSES >= 3:
                        passes.append((wih_lo_sb, xh))
                ps = xgps.tile([128, WIN, B], F32, tag="xgps")
                for pi, (wt, xt_) in enumerate(passes):
                    for k in range(DC):
                        nc.tensor.matmul(
                            ps[:],
                            wt[:, k, m * 128:(m + 1) * 128],
                            xt_[:, k, :, :],
                            start=(pi == 0 and k == 0),
                            stop=(pi == len(passes) - 1 and k == DC - 1),
                            skip_group_check=True)
                return ps

            def proj_copy(w, ps, xg, m):
                warm = w < NWARM
                bsb = bias_w_sb if warm else bias_m_sb
                if m % 2:
                    nc.scalar.activation(xg[:, m, :, :], ps[:], AF.Identity,
                                         bias=bsb[:, m:m + 1], scale=1.0)
                else:
                    bs_ap = bsb[:, m:m + 1]
                    bias_b = bass.AP(tensor=bs_ap.tensor, offset=bs_ap.offset,
                                     ap=[list(bs_ap.ap[0]), [0, WIN], [0, B]])
                    nc.vector.tensor_add(xg[:, m, :, :], ps[:], bias_b)

            def proj_mgroup(w, xh, xl, xg, m):
                proj_copy(w, proj_mm(w, xh, xl, m), xg, m)

            # prologue: window 0 projected up-front; thereafter window w+1's
            # projection m-groups are spread between window w's steps so PE
            # always has h16-independent work while gate chains drain
            MG_SCHED = [2, 1, 2, 1, 2, 1, 2, 1]
            xh, xl = dma_window(0)
            xg = xgp.tile([128, GC, WIN, B], F32, tag="xg")
            for m in range(GC):
                proj_mgroup(0, xh, xl, xg, m)

            for w in range(NW):
                if w + 1 < NW:
                    nxh, nxl = dma_window(w + 1)
                    nxg = xgp.tile([128, GC, WIN, B], F32, tag="xg")
                    mg_next = 0
                    pend = []  # (ps, m, issue_step): copy 2 steps later
                for t in range(WIN):
                    ts = w * WIN + t
                    h16_prev = h16
                    h16 = h16p.tile([128, HC, B], F16, tag="h16")
                    gts = []
                    for c in range(HC):
                        gt_ = gps.tile([128, 4, B], F32, tag=f"g{c}",
                                       name=f"gt{c}")
                        for k in range(HC):
                            for mi in range(4):
                                m = 4 * c + mi
                                nc.tensor.matmul(
                                    gt_[:, mi, :],
                                    whh_sb[:, k, m * 128:(m + 1) * 128],
                                    h16_prev[:, k, :],
                                    start=(k == 0 and mi == 0),
                                    stop=(k == HC - 1),
                                    skip_group_check=True)
                        gts.append(gt_)
                    # emission for the PREVIOUS step's h16 (fully ready)
                    if ts >= W + 1:
                        t0 = ts - 1 - W
                        pe_em = emps.tile([B, T], F32, tag="em")
                        for k in range(HC):
                            nc.tensor.matmul(pe_em[:], h16_prev[:, k, :],
                                             weff_sb[:, k, :],
                                             start=(k == 0),
                                             stop=(k == HC - 1),
                                             skip_group_check=True)
                        off = t0 + pb * (CH - 1 - 2 * t0)
                        nc.vector.tensor_copy(
                            emT_sb[:, bass.ds(off, 1), :], pe_em[:])
                    for c in range(HC):
                        # gate tiles per chunk: [i, f, o, g]
                        gsb = gtp.tile([128, 4, B], F32, tag="gsb")
                        nc.vector.tensor_add(gsb[:], gts[c][:],
                                             xg[:, 4 * c:4 * c + 4, t, :])
                        sio = gtp.tile([128, 3, B], F32, tag="sio")
                        nc.scalar.activation(sio[:], gsb[:, 0:3, :],
                                             AF.Sigmoid)
                        tg = gtp.tile([128, 1, B], F32, tag="tg")
                        nc.scalar.activation(tg[:], gsb[:, 3:4, :], AF.Tanh)
                        t1 = gtp.tile([128, 1, B], F32, tag="t1")
                        nc.vector.tensor_mul(t1[:, 0, :], sio[:, 1, :],
                                             c32[:, c, :])
                        t2 = gtp.tile([128, 1, B], F32, tag="t2")
                        nc.vector.tensor_mul(t2[:, 0, :], sio[:, 0, :],
                                             tg[:, 0, :])
                        nc.vector.tensor_add(c32[:, c, :], t1[:, 0, :],
                                             t2[:, 0, :])
                        tc_ = gtp.tile([128, 1, B], F32, tag="tc_")
                        nc.scalar.activation(tc_[:], c32[:, c:c + 1, :],
                                             AF.Tanh)
                        nc.vector.tensor_mul(h16[:, c, :], sio[:, 2, :],
                                             tc_[:, 0, :])
                    if w + 1 < NW:
                        for _ in range(MG_SCHED[t]):
                            if mg_next < GC:
                                ps = proj_mm(w + 1, nxh, nxl, mg_next)
                                pend.append((ps, mg_next, t))
                                mg_next += 1
                        while pend and (pend[0][2] <= t - 1 or t == WIN - 1):
                            ps, m_, _ = pend.pop(0)
                            proj_copy(w + 1, ps, nxg, m_)
                if w + 1 < NW:
                    while pend:
                        ps, m_, _ = pend.pop(0)
                        proj_copy(w + 1, ps, nxg, m_)
                    xh, xl, xg = nxh, nxl, nxg
            # final step's emission
            pe_em = emps.tile([B, T], F32, tag="em")
            for c in range(HC):
                nc.tensor.matmul(pe_em[:], h16[:, c, :], weff_sb[:, c, :],
                                 start=(c == 0), stop=(c == HC - 1),
                                 skip_group_check=True)
            t0 = CH - 1
            off = t0 + pb * (CH - 1 - 2 * t0)
            nc.vector.tensor_copy(emT_sb[:, bass.ds(off, 1), :], pe_em[:])
            nc.sync.dma_start(emT_bounce.ap()[:], emT_sb[:])

        # ---------- phase C: allgather ----------
        nc.gpsimd.collective_compute(
            "AllGather", ALU.bypass,
            replica_groups=[list(range(8))],
            ins=[emT_bounce.ap()],
            outs=[emg_bounce.ap()],
        )

        # ---------- phase D: viterbi ----------
        with tc.tile_pool(name="vg", bufs=1) as vgp, \
             tc.tile_pool(name="vt", bufs=4) as vtp, \
             tc.tile_pool(name="vh", bufs=1) as vhp:
            emg_sb = vgp.tile([B, 8, CH, T], F16)
            nc.sync.dma_start(emg_sb[:],
                              emg_bounce.ap().rearrange("c b s t -> b c s t"))
            masknot_sb = vgp.tile([B, PADW], U8)
            nc.sync.dma_start(masknot_sb[:], masknot)

            em_full = vgp.tile([B, PADW, T], F32)
            nc.vector.memset(em_full[:, 0:VW, :], 0.0)
            nc.vector.memset(em_full[:, VW + S:, :], 0.0)
            for g in range(4):
                nc.vector.tensor_add(
                    em_full[:, VW + CH * g:VW + CH * (g + 1), :],
                    emg_sb[:, g, :, :],
                    emg_sb[:, 7 - g, :, :])

            vbase = nc.partition_id() * VCH

            hist = vhp.tile([B, SPAN_V, T], F32)
            aspan = vhp.tile([B, SPAN_V, T, T], F32)
            emf_sl = em_full[:, bass.ds(vbase, SPAN_V), :]
            tr_b = bass.AP(tensor=transb_sb.tensor,
                           offset=transb_sb[:].offset,
                           ap=[list(transb_sb[:].ap[0]), [0, SPAN_V], [T, T],
                               [1, T]])
            dg_ap = diagm_sb[:]
            dg_b = bass.AP(tensor=dg_ap.tensor, offset=dg_ap.offset,
                           ap=[list(dg_ap.ap[0]), [0, SPAN_V], [T, T],
                               [1, T]])
            mk_sp = vgp.tile([B, SPAN_V], U8)
            nc.vector.tensor_copy(mk_sp[:],
                                  masknot_sb[:, bass.ds(vbase, SPAN_V)])
            mk_ap2 = mk_sp[:]
            mk_b2 = bass.AP(tensor=mk_ap2.tensor, offset=mk_ap2.offset,
                            ap=[list(mk_ap2.ap[0]), [1, SPAN_V], [0, T],
                                [0, T]])

            # build aspan in halves: gpsimd add, DVE mask-predication
            HALF = SPAN_V // 2
            for lo in (0, HALF):
                hi = lo + HALF
                em_b = bass.AP(tensor=emf_sl.tensor,
                               offset=emf_sl.offset + lo * T,
                               ap=[list(emf_sl.ap[0]), [T, HALF], [1, T],
                                   [0, T]])
                trb_h = bass.AP(tensor=tr_b.tensor, offset=tr_b.offset,
                                ap=[list(tr_b.ap[0]), [0, HALF], [T, T],
                                    [1, T]])
                dgb_h = bass.AP(tensor=dg_b.tensor, offset=dg_b.offset,
                                ap=[list(dg_b.ap[0]), [0, HALF], [T, T],
                                    [1, T]])
                mkb_h = bass.AP(tensor=mk_b2.tensor, offset=mk_b2.offset + lo,
                                ap=[list(mk_b2.ap[0]), [1, HALF], [0, T],
                                    [0, T]])
                nc.vector.tensor_add(aspan[:, lo:hi, :, :], em_b, trb_h)
                nc.vector.copy_predicated(aspan[:, lo:hi, :, :], mkb_h,
                                          dgb_h)

            score = vtp.tile([B, T], F32, tag="vscore")
            t0v = vtp.tile([B, T], F32, tag="vt0")
            nc.vector.tensor_mul(t0v[:], vinitsel_sb[:],
                                 em_full[:, bass.ds(vbase + VW, 1), :])
            nc.vector.tensor_add(score[:], t0v[:], vinit_sb[:])

            for li in range(1, SPAN_V):
                tmp = vtp.tile([B, T, T], F32, tag="vtmp")
                nc.vector.tensor_add(tmp[:], aspan[:, li, :, :],
                                     _bc(score[:], T, 0))
                score = vtp.tile([B, T], F32, tag="vscore")
                nc.vector.reduce_max(score[:], tmp[:],
                                     axis=mybir.AxisListType.X)
                eq = vtp.tile([B, T, T], F32, tag="veq")
                nc.vector.tensor_tensor(eq[:], tmp[:], _bc(score[:], T, 1),
                                        op=ALU.is_equal)
                idm = vtp.tile([B, T, T], F32, tag="vidm")
                nc.gpsimd.tensor_mul(idm[:], eq[:], iota81_sb[:])
                nc.vector.reduce_max(hist[:, li, :], idm[:],
                                     axis=mybir.AxisListType.X)

            nc.vector.tensor_add(score[:], score[:], end9_sb[:])
            mxf = vtp.tile([B, 1], F32, tag="vmxf")
            nc.vector.reduce_max(mxf[:], score[:], axis=mybir.AxisListType.X)
            eqf = vtp.tile([B, T], F32, tag="veqf")
            nc.vector.tensor_tensor(eqf[:], score[:], _bccol(mxf[:], T),
                                    op=ALU.is_equal)
            idf = vtp.tile([B, T], F32, tag="vidf")
            nc.vector.tensor_mul(idf[:], eqf[:], iota9_sb[:])
            tag = vgp.tile([B, 1], F32)
            nc.vector.reduce_max(tag[:], idf[:], axis=mybir.AxisListType.X)

            tags_f = vgp.tile([B, VCH], F32)
            for li in range(SPAN_V - 1, VW - 1, -1):
                if VW <= li < VW + VCH:
                    nc.vector.tensor_copy(tags_f[:, li - VW:li - VW + 1],
                                          tag[:])
                if li == VW:
                    break
                oh = vtp.tile([B, T], F32, tag="voh")
                nc.vector.tensor_tensor(oh[:], iota9_sb[:], _bccol(tag[:], T),
                                        op=ALU.is_equal)
                ohh = vtp.tile([B, T], F32, tag="vohh")
                nc.vector.tensor_mul(ohh[:], oh[:], hist[:, li, :])
                nc.vector.reduce_max(tag[:], ohh[:],
                                     axis=mybir.AxisListType.X)

            tags_i = vgp.tile([B, VCH], I32)
            nc.vector.tensor_copy(tags_i[:], tags_f[:])
            nc.sync.dma_start(tags_out[:], tags_i[:])

    nc.compile()
    return nc


def _host_prep(inputs):
    f32 = np.float32
    bert = np.asarray(inputs["bert_out"], f32)
    mask = np.asarray(inputs["mask"]).astype(bool)

    # gate-tile order per chunk: [i, f, o, g]  (pytorch rows are i,f,g,o)
    qorder = [0, 1, 3, 2]
    perm = np.concatenate([
        np.arange(q * H + c * 128, q * H + (c + 1) * 128)
        for c in range(HC) for q in qorder])

    def split16(x):
        hi = x.astype(np.float16)
        lo = (x.astype(f32) - hi.astype(f32)).astype(np.float16)
        return hi, lo

    dirs = []
    for d, sfx in enumerate(("f", "b")):
        Wih = np.asarray(inputs[f"Wih_{sfx}"], f32)[perm]     # [1536, 768]
        Whh = np.asarray(inputs[f"Whh_{sfx}"], f32)[perm]     # [1536, 384]
        bias = (np.asarray(inputs[f"bih_{sfx}"], f32)
                + np.asarray(inputs[f"bhh_{sfx}"], f32))[perm]
        wihT = np.ascontiguousarray(Wih.T)                    # [768, 1536]
        whhT = np.ascontiguousarray(Whh.T)                    # [384, 1536]
        wh_hi, wh_lo = split16(wihT)
        wih_hi = wh_hi.reshape(DC, 128, 4 * H).transpose(1, 0, 2).copy()
        wih_lo = wh_lo.reshape(DC, 128, 4 * H).transpose(1, 0, 2).copy()
        whh16 = whhT.astype(np.float16).reshape(HC, 128, 4 * H) \
            .transpose(1, 0, 2).copy()
        bias_m = bias.reshape(GC, 128).T.copy()
        dirs.append(dict(wih_hi=wih_hi, wih_lo=wih_lo, whh16=whh16,
                         bias_m=bias_m))

    W1 = np.asarray(inputs["W1"], np.float64)
    W2 = np.asarray(inputs["W2"], np.float64)
    Wc = np.asarray(inputs["Wc"], np.float64)
    b1 = np.asarray(inputs["b1"], np.float64)
    b2 = np.asarray(inputs["b2"], np.float64)
    bc = np.asarray(inputs["bc"], np.float64)
    W_eff = (Wc @ W2 @ W1).astype(f32)                        # [9, 768]
    b_eff = (Wc @ (W2 @ b1 + b2) + bc).astype(f32)

    trans = np.asarray(inputs["trans"], f32)
    start = np.asarray(inputs["start_trans"], f32)
    end = np.asarray(inputs["end_trans"], f32)

    transb = np.ascontiguousarray(trans.T) + b_eff[:, None]   # [j, i] + beff
    diagm = np.where(np.eye(T, dtype=bool), 0.0, -1e9).astype(f32)
    iota81 = np.tile(np.arange(T, dtype=f32), (T, 1))         # [j, i] = i
    iota9 = np.arange(T, dtype=f32)

    masknot = np.ones((B, PADW), np.uint8)
    masknot[:, VW + 1:VW + S] = 1 - mask[:, 1:].astype(np.uint8)

    in_maps = []
    for core in range(8):
        d, ch = core // 4, core % 4
        dd = dirs[d]
        t0 = ch * CH
        lo = t0 - W
        band = np.zeros((B, SPAN, D), f32)
        if d == 0:
            s0 = max(lo, 0)
            band[:, s0 - lo:, :] = bert[:, s0:t0 + CH, :]
        else:
            idx = (S - 1) - np.arange(lo, t0 + CH)
            valid = idx <= S - 1
            band[:, valid, :] = bert[:, idx[valid], :]
        b_hi, b_lo = split16(band)                            # [B, SPAN, D]

        def xt_layout(x16):
            # [B, SPAN, D] -> [128, DC, SPAN, B]
            return np.ascontiguousarray(
                x16.transpose(2, 1, 0).reshape(DC, 128, SPAN, B)
                .transpose(1, 0, 2, 3))

        bias_wv = dd["bias_m"].copy()
        if ch == 0:
            bias_wv[:, [0, 4, 8]] = -30.0   # freeze i-gate during warmup
        weff_half = W_eff[:, d * H:(d + 1) * H].T              # [384, 9]
        weff_t = weff_half.reshape(HC, 128, T).transpose(1, 0, 2) \
            .astype(np.float16).copy()

        end9d = np.broadcast_to(end, (B, T)).copy() if core == 7 \
            else np.zeros((B, T), f32)
        vinitv = np.broadcast_to(start + b_eff, (B, T)).copy() if core == 0 \
            else np.zeros((B, T), f32)
        vinitsel = np.full((B, T), 1.0 if core == 0 else 0.0, f32)

        im = dict(
            xt_hi=xt_layout(b_hi), xt_lo=xt_layout(b_lo),
            wih_hi=dd["wih_hi"], whh16=dd["whh16"],
            bias_w=bias_wv, bias_m=dd["bias_m"],
            weff16=weff_t,
            transb=transb, diagm=diagm, iota81=iota81, iota9=iota9,
            end9d=end9d, vinit=vinitv, vinitsel=vinitsel,
            masknot=masknot,
        )
        if PASSES >= 3:
            im["wih_lo"] = dd["wih_lo"]
        in_maps.append(im)
    return in_maps


def kernel(**inputs):
    global _cache
    if "nc" not in _cache:
        _cache["nc"] = _build()
    nc = _cache["nc"]
    in_maps = _host_prep(inputs)
    res = run_bass_kernel_spmd(nc, in_maps, core_ids=list(range(8)))
    tags = np.concatenate([res.results[c]["tags"] for c in range(8)], axis=1)
    return tags.astype(np.int32)


# revision 40
# speedup vs baseline: 1.0910x; 1.0910x over previous
"""BertLSTMCrf TRN2 kernel: 8-core Bass/Tile implementation (v2).

Sharding: 8 cores = 2 LSTM directions x 4 time-chunks of 128 steps, each
with a 32-step zero-state warmup.  v2 changes vs baseline:
  - bert band transposed + fp16 hi/lo split on HOST -> no PE transposes,
    no on-device casts; x arrives as [128, DC, SPAN, B] fp16 hi/lo.
  - 2-pass projection (w_hi*x_hi + w_hi*x_lo): x is exact in hi+lo,
    weights fp16-rounded (1-pass in the 32-step warmup region).
  - emissions fused into the recurrence: per-step [B,T] matmul with h16
    stationary (b-cores store time-reversed via partition-id addressing),
    so phase B + the h HBM round-trip are gone; b_eff folded into
    trans/vinit on host.
  - viterbi: mask folded into the precomputed aspan (masked steps become
    identity transitions) -> no copy_predicated in the sequential loops.
  - LSTM gate order [i,f,o,g] per chunk -> one fused sigmoid; c-update
    muls/adds on gpsimd to unload DVE.
"""
import numpy as np
from contextlib import ExitStack

import concourse.bass as bass
import concourse.mybir as mybir
import concourse.tile as tile
from concourse import bacc
from concourse.bass_utils import run_bass_kernel_spmd

F32 = mybir.dt.float32
F16 = mybir.dt.float16
I32 = mybir.dt.int32
U8 = mybir.dt.uint8
AF = mybir.ActivationFunctionType
ALU = mybir.AluOpType

B, S, D, H, T = 64, 512, 768, 384, 9
NCH, CH, W = 4, 128, 16
WIN = 8
SPAN = CH + W                    # 160
NW = SPAN // WIN                 # 20
NWARM = W // WIN                 # 4
PASSES = 1                       # 1: hi*hi   2: + hi(w)*lo(x)   3: + lo(w)*hi(x)
VCH, VW = 64, 32
SPAN_V = VW + VCH + VW           # 192
PADW = VW + S + VW               # 640
DC, HC, GC = D // 128, H // 128, 4 * H // 128

_cache = {}


def _bc(ap, n, pos):
    """Insert a broadcast (step-0) free dim of length n at free position."""
    a = [list(x) for x in ap.ap]
    a.insert(1 + pos, [0, n])
    return bass.AP(tensor=ap.tensor, offset=ap.offset, ap=a)


def _bccol(ap, n):
    """Turn a [P,1] column AP into a [P,n] broadcast AP."""
    a = [list(x) for x in ap.ap]
    assert a[-1][1] == 1
    a[-1] = [0, n]
    return bass.AP(tensor=ap.tensor, offset=ap.offset, ap=a)


def _build():
    nc = bacc.Bacc("TRN2", target_bir_lowering=False, debug=False,
                   num_devices=8)
    dt = nc.dram_tensor
    xt_hi = dt("xt_hi", [128, DC, SPAN, B], F16, kind="ExternalInput").ap()
    xt_lo = dt("xt_lo", [128, DC, SPAN, B], F16, kind="ExternalInput").ap()
    wih_hi = dt("wih_hi", [128, DC, 4 * H], F16, kind="ExternalInput").ap()
    if PASSES >= 3:
        wih_lo = dt("wih_lo", [128, DC, 4 * H], F16,
                    kind="ExternalInput").ap()
    whh16 = dt("whh16", [128, HC, 4 * H], F16, kind="ExternalInput").ap()
    bias_w = dt("bias_w", [128, GC], F32, kind="ExternalInput").ap()
    bias_m = dt("bias_m", [128, GC], F32, kind="ExternalInput").ap()
    weff16 = dt("weff16", [128, HC, T], F16, kind="ExternalInput").ap()
    transb = dt("transb", [T, T], F32, kind="ExternalInput").ap()
    diagm = dt("diagm", [T, T], F32, kind="ExternalInput").ap()
    iota81 = dt("iota81", [T, T], F32, kind="ExternalInput").ap()
    iota9 = dt("iota9", [T], F32, kind="ExternalInput").ap()
    end9d = dt("end9d", [B, T], F32, kind="ExternalInput").ap()
    vinit = dt("vinit", [B, T], F32, kind="ExternalInput").ap()
    vinitsel = dt("vinitsel", [B, T], F32, kind="ExternalInput").ap()
    masknot = dt("masknot", [B, PADW], U8, kind="ExternalInput").ap()

    emT_bounce = dt("emT_bounce", [B, CH, T], F16)
    emg_bounce = dt("emg_bounce", [8, B, CH, T], F16)

    tags_out = dt("tags", [B, VCH], I32, kind="ExternalOutput").ap()

    with tile.TileContext(nc) as tc, ExitStack() as ctx:
        cpool = ctx.enter_context(tc.tile_pool(name="consts", bufs=1))
        wih_hi_sb = cpool.tile([128, DC, 4 * H], F16)
        nc.sync.dma_start(wih_hi_sb[:], wih_hi)
        if PASSES >= 3:
            wih_lo_sb = cpool.tile([128, DC, 4 * H], F16)
            nc.sync.dma_start(wih_lo_sb[:], wih_lo)
        whh_sb = cpool.tile([128, HC, 4 * H], F16)
        nc.sync.dma_start(whh_sb[:], whh16)
        bias_w_sb = cpool.tile([128, GC], F32)
        bias_m_sb = cpool.tile([128, GC], F32)
        nc.sync.dma_start(bias_w_sb[:], bias_w)
        nc.sync.dma_start(bias_m_sb[:], bias_m)
        weff_sb = cpool.tile([128, HC, T], F16)
        nc.sync.dma_start(weff_sb[:], weff16)

        def dma_bcast(dst, src):
            p = dst.shape[0]
            src_b = bass.AP(tensor=src.tensor, offset=src.offset,
                            ap=[[0, p]] + [list(x) for x in src.ap])
            nc.sync.dma_start(dst[:], src_b)

        transb_sb = cpool.tile([B, T, T], F32)
        dma_bcast(transb_sb, transb)
        diagm_sb = cpool.tile([B, T, T], F32)
        dma_bcast(diagm_sb, diagm)
        iota81_sb = cpool.tile([B, T, T], F32)
        dma_bcast(iota81_sb, iota81)
        iota9_sb = cpool.tile([B, T], F32)
        dma_bcast(iota9_sb, iota9)
        end9_sb = cpool.tile([B, T], F32)
        nc.sync.dma_start(end9_sb[:], end9d)
        vinit_sb = cpool.tile([B, T], F32)
        nc.sync.dma_start(vinit_sb[:], vinit)
        vinitsel_sb = cpool.tile([B, T], F32)
        nc.sync.dma_start(vinitsel_sb[:], vinitsel)

        # b-cores (partition_id >= 4) store emissions time-reversed so the
        # allgathered chunks line up forward; em index = t0 + pb*(CH-1-2*t0)
        pb = nc.partition_id() // 4

        # ---------- phase A: projection + recurrence + fused emission ----
        with tc.tile_pool(name="xt", bufs=2) as xtp, \
             tc.tile_pool(name="xg", bufs=2) as xgp, \
             tc.tile_pool(name="st", bufs=1) as stp, \
             tc.tile_pool(name="gt", bufs=4) as gtp, \
             tc.tile_pool(name="h16p", bufs=2) as h16p, \
             tc.tile_pool(name="emsb", bufs=1) as emp, \
             tc.tile_pool(name="xgps", bufs=3, space="PSUM") as xgps, \
             tc.tile_pool(name="gps", bufs=1, space="PSUM") as gps, \
             tc.tile_pool(name="emps", bufs=1, space="PSUM") as emps:

            c32 = stp.tile([128, HC, B], F32)
            nc.vector.memset(c32[:], 0.0)
            h16 = h16p.tile([128, HC, B], F16, tag="h16")
            nc.vector.memset(h16[:], 0.0)
            emT_sb = emp.tile([B, CH, T], F16)

            def dma_window(w):
                warm = w < NWARM
                xh = xtp.tile([128, DC, WIN, B], F16, tag="xh")
                nc.sync.dma_start(xh[:],
                                  xt_hi[:, :, w * WIN:(w + 1) * WIN, :])
                xl = None
                if not warm and PASSES >= 2:
                    xl = xtp.tile([128, DC, WIN, B], F16, tag="xl")
                    nc.sync.dma_start(xl[:],
                                      xt_lo[:, :, w * WIN:(w + 1) * WIN, :])
                return xh, xl

            def proj_mm(w, xh, xl, m):
                warm = w < NWARM
                passes = [(wih_hi_sb, xh)]
                if not warm:
                    if PASSES >= 2:
                        passes.append((wih_hi_sb, xl))
                    if PASSES >= 3:
                        passes.append((wih_lo_sb, xh))
                ps = xgps.tile([128, WIN, B], F32, tag="xgps")
                for pi, (wt, xt_) in enumerate(passes):
                    for k in range(DC):
                        nc.tensor.matmul(
                            ps[:],
                            wt[:, k, m * 128:(m + 1) * 128],
                            xt_[:, k, :, :],
                            start=(pi == 0 and k == 0),
                            stop=(pi == len(passes) - 1 and k == DC - 1),
                            skip_group_check=True)
                return ps

            def proj_copy(w, ps, xg, m):
                warm = w < NWARM
                bsb = bias_w_sb if warm else bias_m_sb
                bs_ap = bsb[:, m:m + 1]
                bias_b = bass.AP(tensor=bs_ap.tensor, offset=bs_ap.offset,
                                 ap=[list(bs_ap.ap[0]), [0, WIN], [0, B]])
                nc.vector.tensor_add(xg[:, m, :, :], ps[:], bias_b)

            def proj_mgroup(w, xh, xl, xg, m):
                proj_copy(w, proj_mm(w, xh, xl, m), xg, m)

            for w in range(NW):
                xh, xl = dma_window(w)
                xg = xgp.tile([128, GC, WIN, B], F32, tag="xg")
                for m in range(GC):
                    proj_mgroup(w, xh, xl, xg, m)
                for t in range(WIN):
                    ts = w * WIN + t
                    h16_prev = h16
                    h16 = h16p.tile([128, HC, B], F16, tag="h16")
                    gts = [gps.tile([128, 4, B], F32, tag=f"g{c}",
                                    name=f"gt{c}")
                           for c in range(HC)]
                    em_on = ts >= W + 1
                    if em_on:
                        t0 = ts - 1 - W
                        pe_em = emps.tile([B, T], F32, tag="em")
                    # k-outer: (group, k) matmul depends only on h16_prev[k]
                    for k in range(HC):
                        for c in range(HC):
                            for mi in range(4):
                                m = 4 * c + mi
                                nc.tensor.matmul(
                                    gts[c][:, mi, :],
                                    whh_sb[:, k, m * 128:(m + 1) * 128],
                                    h16_prev[:, k, :],
                                    start=(k == 0 and mi == 0),
                                    stop=(k == HC - 1),
                                    skip_group_check=True)
                        if em_on:
                            nc.tensor.matmul(pe_em[:], h16_prev[:, k, :],
                                             weff_sb[:, k, :],
                                             start=(k == 0),
                                             stop=(k == HC - 1),
                                             skip_group_check=True)
                    if em_on:
                        off = t0 + pb * (CH - 1 - 2 * t0)
                        nc.vector.tensor_copy(
                            emT_sb[:, bass.ds(off, 1), :], pe_em[:])
                    for c in range(HC):
                        # gate tiles per chunk: [i, f, o, g]
                        gsb = gtp.tile([128, 4, B], F32, tag="gsb")
                        nc.vector.tensor_add(gsb[:], gts[c][:],
                                             xg[:, 4 * c:4 * c + 4, t, :])
                        sio = gtp.tile([128, 3, B], F32, tag="sio")
                        nc.scalar.activation(sio[:], gsb[:, 0:3, :],
                                             AF.Sigmoid)
                        tg = gtp.tile([128, 1, B], F32, tag="tg")
                        nc.scalar.activation(tg[:], gsb[:, 3:4, :], AF.Tanh)
                        t1 = gtp.tile([128, 1, B], F32, tag="t1")
                        nc.gpsimd.tensor_mul(t1[:, 0, :], sio[:, 1, :],
                                             c32[:, c, :])
                        t2 = gtp.tile([128, 1, B], F32, tag="t2")
                        nc.gpsimd.tensor_mul(t2[:, 0, :], sio[:, 0, :],
                                             tg[:, 0, :])
                        nc.gpsimd.tensor_add(c32[:, c, :], t1[:, 0, :],
                                             t2[:, 0, :])
                        tc_ = gtp.tile([128, 1, B], F32, tag="tc_")
                        nc.scalar.activation(tc_[:], c32[:, c:c + 1, :],
                                             AF.Tanh)
                        nc.vector.tensor_mul(h16[:, c, :], sio[:, 2, :],
                                             tc_[:, 0, :])
            # final step's emission
            pe_em = emps.tile([B, T], F32, tag="em")
            for c in range(HC):
                nc.tensor.matmul(pe_em[:], h16[:, c, :], weff_sb[:, c, :],
                                 start=(c == 0), stop=(c == HC - 1),
                                 skip_group_check=True)
            t0 = CH - 1
            off = t0 + pb * (CH - 1 - 2 * t0)
            nc.vector.tensor_copy(emT_sb[:, bass.ds(off, 1), :], pe_em[:])
            nc.sync.dma_start(emT_bounce.ap()[:], emT_sb[:])

        # ---------- phase C: allgather ----------
        nc.gpsimd.collective_compute(
            "AllGather", ALU.bypass,
            replica_groups=[list(range(8))],
            ins=[emT_bounce.ap()],
            outs=[emg_bounce.ap()],
        )

        # ---------- phase D: viterbi ----------
        with tc.tile_pool(name="vg", bufs=1) as vgp, \
             tc.tile_pool(name="vt", bufs=4) as vtp, \
             tc.tile_pool(name="vh", bufs=1) as vhp:
            emg_sb = vgp.tile([B, 8, CH, T], F16)
            nc.sync.dma_start(emg_sb[:],
                              emg_bounce.ap().rearrange("c b s t -> b c s t"))
            masknot_sb = vgp.tile([B, PADW], U8)
            nc.sync.dma_start(masknot_sb[:], masknot)

            em_full = vgp.tile([B, PADW, T], F32)
            nc.vector.memset(em_full[:, 0:VW, :], 0.0)
            nc.vector.memset(em_full[:, VW + S:, :], 0.0)
            for g in range(4):
                nc.vector.tensor_add(
                    em_full[:, VW + CH * g:VW + CH * (g + 1), :],
                    emg_sb[:, g, :, :],
                    emg_sb[:, 7 - g, :, :])

            vbase = nc.partition_id() * VCH

            hist = vhp.tile([B, SPAN_V, T], F32)
            aspan = vhp.tile([B, SPAN_V, T, T], F32)
            emf_sl = em_full[:, bass.ds(vbase, SPAN_V), :]
            tr_b = bass.AP(tensor=transb_sb.tensor,
                           offset=transb_sb[:].offset,
                           ap=[list(transb_sb[:].ap[0]), [0, SPAN_V], [T, T],
                               [1, T]])
            dg_ap = diagm_sb[:]
            dg_b = bass.AP(tensor=dg_ap.tensor, offset=dg_ap.offset,
                           ap=[list(dg_ap.ap[0]), [0, SPAN_V], [T, T],
                               [1, T]])
            mk_sp = vgp.tile([B, SPAN_V], U8)
            nc.vector.tensor_copy(mk_sp[:],
                                  masknot_sb[:, bass.ds(vbase, SPAN_V)])
            mk_ap2 = mk_sp[:]
            mk_b2 = bass.AP(tensor=mk_ap2.tensor, offset=mk_ap2.offset,
                            ap=[list(mk_ap2.ap[0]), [1, SPAN_V], [0, T],
                                [0, T]])

            # build aspan in halves: gpsimd add, DVE mask-predication
            HALF = SPAN_V // 2
            for lo in (0, HALF):
                hi = lo + HALF
                em_b = bass.AP(tensor=emf_sl.tensor,
                               offset=emf_sl.offset + lo * T,
                               ap=[list(emf_sl.ap[0]), [T, HALF], [1, T],
                                   [0, T]])
                trb_h = bass.AP(tensor=tr_b.tensor, offset=tr_b.offset,
                                ap=[list(tr_b.ap[0]), [0, HALF], [T, T],
                                    [1, T]])
                dgb_h = bass.AP(tensor=dg_b.tensor, offset=dg_b.offset,
                                ap=[list(dg_b.ap[0]), [0, HALF], [T, T],
                                    [1, T]])
                mkb_h = bass.AP(tensor=mk_b2.tensor, offset=mk_b2.offset + lo,
                                ap=[list(mk_b2.ap[0]), [1, HALF], [0, T],
                                    [0, T]])
                nc.vector.tensor_add(aspan[:, lo:hi, :, :], em_b, trb_h)
                nc.vector.copy_predicated(aspan[:, lo:hi, :, :], mkb_h,
                                          dgb_h)

            score = vtp.tile([B, T], F32, tag="vscore")
            t0v = vtp.tile([B, T], F32, tag="vt0")
            nc.vector.tensor_mul(t0v[:], vinitsel_sb[:],
                                 em_full[:, bass.ds(vbase + VW, 1), :])
            nc.vector.tensor_add(score[:], t0v[:], vinit_sb[:])

            for li in range(1, SPAN_V):
                tmp = vtp.tile([B, T, T], F32, tag="vtmp")
                nc.vector.tensor_add(tmp[:], aspan[:, li, :, :],
                                     _bc(score[:], T, 0))
                score = vtp.tile([B, T], F32, tag="vscore")
                nc.vector.reduce_max(score[:], tmp[:],
                                     axis=mybir.AxisListType.X)
                eq = vtp.tile([B, T, T], F32, tag="veq")
                nc.vector.tensor_tensor(eq[:], tmp[:], _bc(score[:], T, 1),
                                        op=ALU.is_equal)
                idm = vtp.tile([B, T, T], F32, tag="vidm")
                nc.gpsimd.tensor_mul(idm[:], eq[:], iota81_sb[:])
                nc.vector.reduce_max(hist[:, li, :], idm[:],
                                     axis=mybir.AxisListType.X)

            nc.vector.tensor_add(score[:], score[:], end9_sb[:])
            mxf = vtp.tile([B, 1], F32, tag="vmxf")
            nc.vector.reduce_max(mxf[:], score[:], axis=mybir.AxisListType.X)
            eqf = vtp.tile([B, T], F32, tag="veqf")
            nc.vector.tensor_tensor(eqf[:], score[:], _bccol(mxf[:], T),
                                    op=ALU.is_equal)
            idf = vtp.tile([B, T], F32, tag="vidf")
            nc.vector.tensor_mul(idf[:], eqf[:], iota9_sb[:])
            tag = vgp.tile([B, 1], F32)
            nc.vector.reduce_max(tag[:], idf[:], axis=mybir.AxisListType.X)

            tags_f = vgp.tile([B, VCH], F32)
            for li in range(SPAN_V - 1, VW - 1, -1):
                if VW <= li < VW + VCH:
                    nc.vector.tensor_copy(tags_f[:, li - VW:li - VW + 1],
                                          tag[:])
                if li == VW:
                    break
                oh = vtp.tile([B, T], F32, tag="voh")
                nc.vector.tensor_tensor(oh[:], iota9_sb[:], _bccol(tag[:], T),
                                        op=ALU.is_equal)
                ohh = vtp.tile([B, T], F32, tag="vohh")
                nc.vector.tensor_mul(ohh[:], oh[:], hist[:, li, :])
                nc.vector.reduce_max(tag[:], ohh[:],
                                     axis=mybir.AxisListType.X)

            tags_i = vgp.tile([B, VCH], I32)
            nc.vector.tensor_copy(tags_i[:], tags_f[:])
            nc.sync.dma_start(tags_out[:], tags_i[:])

    nc.compile()
    return nc


def _host_prep(inputs):
    f32 = np.float32
    bert = np.asarray(inputs["bert_out"], f32)
    mask = np.asarray(inputs["mask"]).astype(bool)

    # gate-tile order per chunk: [i, f, o, g]  (pytorch rows are i,f,g,o)
    qorder = [0, 1, 3, 2]
    perm = np.concatenate([
        np.arange(q * H + c * 128, q * H + (c + 1) * 128)
        for c in range(HC) for q in qorder])

    def split16(x):
        hi = x.astype(np.float16)
        lo = (x.astype(f32) - hi.astype(f32)).astype(np.float16)
        return hi, lo

    dirs = []
    for d, sfx in enumerate(("f", "b")):
        Wih = np.asarray(inputs[f"Wih_{sfx}"], f32)[perm]     # [1536, 768]
        Whh = np.asarray(inputs[f"Whh_{sfx}"], f32)[perm]     # [1536, 384]
        bias = (np.asarray(inputs[f"bih_{sfx}"], f32)
                + np.asarray(inputs[f"bhh_{sfx}"], f32))[perm]
        wihT = np.ascontiguousarray(Wih.T)                    # [768, 1536]
        whhT = np.ascontiguousarray(Whh.T)                    # [384, 1536]
        wh_hi, wh_lo = split16(wihT)
        wih_hi = wh_hi.reshape(DC, 128, 4 * H).transpose(1, 0, 2).copy()
        wih_lo = wh_lo.reshape(DC, 128, 4 * H).transpose(1, 0, 2).copy()
        whh16 = whhT.astype(np.float16).reshape(HC, 128, 4 * H) \
            .transpose(1, 0, 2).copy()
        bias_m = bias.reshape(GC, 128).T.copy()
        dirs.append(dict(wih_hi=wih_hi, wih_lo=wih_lo, whh16=whh16,
                         bias_m=bias_m))

    W1 = np.asarray(inputs["W1"], np.float64)
    W2 = np.asarray(inputs["W2"], np.float64)
    Wc = np.asarray(inputs["Wc"], np.float64)
    b1 = np.asarray(inputs["b1"], np.float64)
    b2 = np.asarray(inputs["b2"], np.float64)
    bc = np.asarray(inputs["bc"], np.float64)
    W_eff = (Wc @ W2 @ W1).astype(f32)                        # [9, 768]
    b_eff = (Wc @ (W2 @ b1 + b2) + bc).astype(f32)

    trans = np.asarray(inputs["trans"], f32)
    start = np.asarray(inputs["start_trans"], f32)
    end = np.asarray(inputs["end_trans"], f32)

    transb = np.ascontiguousarray(trans.T) + b_eff[:, None]   # [j, i] + beff
    diagm = np.where(np.eye(T, dtype=bool), 0.0, -1e9).astype(f32)
    iota81 = np.tile(np.arange(T, dtype=f32), (T, 1))         # [j, i] = i
    iota9 = np.arange(T, dtype=f32)

    masknot = np.ones((B, PADW), np.uint8)
    masknot[:, VW + 1:VW + S] = 1 - mask[:, 1:].astype(np.uint8)

    in_maps = []
    for core in range(8):
        d, ch = core // 4, core % 4
        dd = dirs[d]
        t0 = ch * CH
        lo = t0 - W
        band = np.zeros((B, SPAN, D), f32)
        if d == 0:
            s0 = max(lo, 0)
            band[:, s0 - lo:, :] = bert[:, s0:t0 + CH, :]
        else:
            idx = (S - 1) - np.arange(lo, t0 + CH)
            valid = idx <= S - 1
            band[:, valid, :] = bert[:, idx[valid], :]
        b_hi, b_lo = split16(band)                            # [B, SPAN, D]

        def xt_layout(x16):
            # [B, SPAN, D] -> [128, DC, SPAN, B]
            return np.ascontiguousarray(
                x16.transpose(2, 1, 0).reshape(DC, 128, SPAN, B)
                .transpose(1, 0, 2, 3))

        bias_wv = dd["bias_m"].copy()
        if ch == 0:
            bias_wv[:, [0, 4, 8]] = -30.0   # freeze i-gate during warmup
        weff_half = W_eff[:, d * H:(d + 1) * H].T              # [384, 9]
        weff_t = weff_half.reshape(HC, 128, T).transpose(1, 0, 2) \
            .astype(np.float16).copy()

        end9d = np.broadcast_to(end, (B, T)).copy() if core == 7 \
            else np.zeros((B, T), f32)
        vinitv = np.broadcast_to(start + b_eff, (B, T)).copy() if core == 0 \
            else np.zeros((B, T), f32)
        vinitsel = np.full((B, T), 1.0 if core == 0 else 0.0, f32)

        im = dict(
            xt_hi=xt_layout(b_hi), xt_lo=xt_layout(b_lo),
            wih_hi=dd["wih_hi"], whh16=dd["whh16"],
            bias_w=bias_wv, bias_m=dd["bias_m"],
            weff16=weff_t,
            transb=transb, diagm=diagm, iota81=iota81, iota9=iota9,
            end9d=end9d, vinit=vinitv, vinitsel=vinitsel,
            masknot=masknot,
        )
        if PASSES >= 3:
            im["wih_lo"] = dd["wih_lo"]
        in_maps.append(im)
    return in_maps


def kernel(**inputs):
    global _cache
    if "nc" not in _cache:
        _cache["nc"] = _build()
    nc = _cache["nc"]
    in_maps = _host_prep(inputs)
    res = run_bass_kernel_spmd(nc, in_maps, core_ids=list(range(8)))
    tags = np.concatenate([res.results[c]["tags"] for c in range(8)], axis=1)
    return tags.astype(np.int32)


# revision 41
# speedup vs baseline: 1.1538x; 1.0576x over previous
"""BertLSTMCrf TRN2 kernel: 8-core Bass/Tile implementation (v2).

Sharding: 8 cores = 2 LSTM directions x 4 time-chunks of 128 steps, each
with a 32-step zero-state warmup.  v2 changes vs baseline:
  - bert band transposed + fp16 hi/lo split on HOST -> no PE transposes,
    no on-device casts; x arrives as [128, DC, SPAN, B] fp16 hi/lo.
  - 2-pass projection (w_hi*x_hi + w_hi*x_lo): x is exact in hi+lo,
    weights fp16-rounded (1-pass in the 32-step warmup region).
  - emissions fused into the recurrence: per-step [B,T] matmul with h16
    stationary (b-cores store time-reversed via partition-id addressing),
    so phase B + the h HBM round-trip are gone; b_eff folded into
    trans/vinit on host.
  - viterbi: mask folded into the precomputed aspan (masked steps become
    identity transitions) -> no copy_predicated in the sequential loops.
  - LSTM gate order [i,f,o,g] per chunk -> one fused sigmoid; c-update
    muls/adds on gpsimd to unload DVE.
"""
import numpy as np
from contextlib import ExitStack

import concourse.bass as bass
import concourse.mybir as mybir
import concourse.tile as tile
from concourse import bacc
from concourse.bass_utils import run_bass_kernel_spmd

F32 = mybir.dt.float32
F16 = mybir.dt.float16
I32 = mybir.dt.int32
U8 = mybir.dt.uint8
AF = mybir.ActivationFunctionType
ALU = mybir.AluOpType

B, S, D, H, T = 64, 512, 768, 384, 9
NCH, CH, W = 4, 128, 8
WIN = 8
SPAN = CH + W                    # 160
NW = SPAN // WIN                 # 20
NWARM = W // WIN                 # 4
PASSES = 1                       # 1: hi*hi   2: + hi(w)*lo(x)   3: + lo(w)*hi(x)
VCH, VW = 64, 32
SPAN_V = VW + VCH + VW           # 192
PADW = VW + S + VW               # 640
DC, HC, GC = D // 128, H // 128, 4 * H // 128

_cache = {}


def _bc(ap, n, pos):
    """Insert a broadcast (step-0) free dim of length n at free position."""
    a = [list(x) for x in ap.ap]
    a.insert(1 + pos, [0, n])
    return bass.AP(tensor=ap.tensor, offset=ap.offset, ap=a)


def _bccol(ap, n):
    """Turn a [P,1] column AP into a [P,n] broadcast AP."""
    a = [list(x) for x in ap.ap]
    assert a[-1][1] == 1
    a[-1] = [0, n]
    return bass.AP(tensor=ap.tensor, offset=ap.offset, ap=a)


def _build():
    nc = bacc.Bacc("TRN2", target_bir_lowering=False, debug=False,
                   num_devices=8)
    dt = nc.dram_tensor
    xt_hi = dt("xt_hi", [128, DC, SPAN, B], F16, kind="ExternalInput").ap()
    xt_lo = dt("xt_lo", [128, DC, SPAN, B], F16, kind="ExternalInput").ap()
    wih_hi = dt("wih_hi", [128, DC, 4 * H], F16, kind="ExternalInput").ap()
    if PASSES >= 3:
        wih_lo = dt("wih_lo", [128, DC, 4 * H], F16,
                    kind="ExternalInput").ap()
    whh16 = dt("whh16", [128, HC, 4 * H], F16, kind="ExternalInput").ap()
    bias_w = dt("bias_w", [128, GC], F32, kind="ExternalInput").ap()
    bias_m = dt("bias_m", [128, GC], F32, kind="ExternalInput").ap()
    weff16 = dt("weff16", [128, HC, T], F16, kind="ExternalInput").ap()
    transb = dt("transb", [T, T], F32, kind="ExternalInput").ap()
    diagm = dt("diagm", [T, T], F32, kind="ExternalInput").ap()
    iota81 = dt("iota81", [T, T], F32, kind="ExternalInput").ap()
    iota9 = dt("iota9", [T], F32, kind="ExternalInput").ap()
    end9d = dt("end9d", [B, T], F32, kind="ExternalInput").ap()
    vinit = dt("vinit", [B, T], F32, kind="ExternalInput").ap()
    vinitsel = dt("vinitsel", [B, T], F32, kind="ExternalInput").ap()
    masknot = dt("masknot", [B, PADW], U8, kind="ExternalInput").ap()

    emT_bounce = dt("emT_bounce", [B, CH, T], F16)
    emg_bounce = dt("emg_bounce", [8, B, CH, T], F16)

    tags_out = dt("tags", [B, VCH], I32, kind="ExternalOutput").ap()

    with tile.TileContext(nc) as tc, ExitStack() as ctx:
        cpool = ctx.enter_context(tc.tile_pool(name="consts", bufs=1))
        wih_hi_sb = cpool.tile([128, DC, 4 * H], F16)
        nc.sync.dma_start(wih_hi_sb[:], wih_hi)
        if PASSES >= 3:
            wih_lo_sb = cpool.tile([128, DC, 4 * H], F16)
            nc.sync.dma_start(wih_lo_sb[:], wih_lo)
        whh_sb = cpool.tile([128, HC, 4 * H], F16)
        nc.sync.dma_start(whh_sb[:], whh16)
        bias_w_sb = cpool.tile([128, GC], F32)
        bias_m_sb = cpool.tile([128, GC], F32)
        nc.sync.dma_start(bias_w_sb[:], bias_w)
        nc.sync.dma_start(bias_m_sb[:], bias_m)
        weff_sb = cpool.tile([128, HC, T], F16)
        nc.sync.dma_start(weff_sb[:], weff16)

        def dma_bcast(dst, src):
            p = dst.shape[0]
            src_b = bass.AP(tensor=src.tensor, offset=src.offset,
                            ap=[[0, p]] + [list(x) for x in src.ap])
            nc.sync.dma_start(dst[:], src_b)

        transb_sb = cpool.tile([B, T, T], F32)
        dma_bcast(transb_sb, transb)
        diagm_sb = cpool.tile([B, T, T], F32)
        dma_bcast(diagm_sb, diagm)
        iota81_sb = cpool.tile([B, T, T], F32)
        dma_bcast(iota81_sb, iota81)
        iota9_sb = cpool.tile([B, T], F32)
        dma_bcast(iota9_sb, iota9)
        end9_sb = cpool.tile([B, T], F32)
        nc.sync.dma_start(end9_sb[:], end9d)
        vinit_sb = cpool.tile([B, T], F32)
        nc.sync.dma_start(vinit_sb[:], vinit)
        vinitsel_sb = cpool.tile([B, T], F32)
        nc.sync.dma_start(vinitsel_sb[:], vinitsel)

        # b-cores (partition_id >= 4) store emissions time-reversed so the
        # allgathered chunks line up forward; em index = t0 + pb*(CH-1-2*t0)
        pb = nc.partition_id() // 4

        # ---------- phase A: projection + recurrence + fused emission ----
        with tc.tile_pool(name="xt", bufs=2) as xtp, \
             tc.tile_pool(name="xg", bufs=2) as xgp, \
             tc.tile_pool(name="st", bufs=1) as stp, \
             tc.tile_pool(name="gt", bufs=4) as gtp, \
             tc.tile_pool(name="h16p", bufs=2) as h16p, \
             tc.tile_pool(name="emsb", bufs=1) as emp, \
             tc.tile_pool(name="xgps", bufs=3, space="PSUM") as xgps, \
             tc.tile_pool(name="gps", bufs=1, space="PSUM") as gps, \
             tc.tile_pool(name="emps", bufs=1, space="PSUM") as emps:

            c32 = stp.tile([128, HC, B], F32)
            nc.vector.memset(c32[:], 0.0)
            h16 = h16p.tile([128, HC, B], F16, tag="h16")
            nc.vector.memset(h16[:], 0.0)
            emT_sb = emp.tile([B, CH, T], F16)

            def dma_window(w):
                warm = w < NWARM
                xh = xtp.tile([128, DC, WIN, B], F16, tag="xh")
                nc.sync.dma_start(xh[:],
                                  xt_hi[:, :, w * WIN:(w + 1) * WIN, :])
                xl = None
                if not warm and PASSES >= 2:
                    xl = xtp.tile([128, DC, WIN, B], F16, tag="xl")
                    nc.sync.dma_start(xl[:],
                                      xt_lo[:, :, w * WIN:(w + 1) * WIN, :])
                return xh, xl

            def proj_mm(w, xh, xl, m):
                warm = w < NWARM
                passes = [(wih_hi_sb, xh)]
                if not warm:
                    if PASSES >= 2:
                        passes.append((wih_hi_sb, xl))
                    if PASSES >= 3:
                        passes.append((wih_lo_sb, xh))
                ps = xgps.tile([128, WIN, B], F32, tag="xgps")
                for pi, (wt, xt_) in enumerate(passes):
                    for k in range(DC):
                        nc.tensor.matmul(
                            ps[:],
                            wt[:, k, m * 128:(m + 1) * 128],
                            xt_[:, k, :, :],
                            start=(pi == 0 and k == 0),
                            stop=(pi == len(passes) - 1 and k == DC - 1),
                            skip_group_check=True)
                return ps

            def proj_copy(w, ps, xg, m):
                warm = w < NWARM
                bsb = bias_w_sb if warm else bias_m_sb
                bs_ap = bsb[:, m:m + 1]
                bias_b = bass.AP(tensor=bs_ap.tensor, offset=bs_ap.offset,
                                 ap=[list(bs_ap.ap[0]), [0, WIN], [0, B]])
                nc.vector.tensor_add(xg[:, m, :, :], ps[:], bias_b)

            def proj_mgroup(w, xh, xl, xg, m):
                proj_copy(w, proj_mm(w, xh, xl, m), xg, m)

            for w in range(NW):
                xh, xl = dma_window(w)
                xg = xgp.tile([128, GC, WIN, B], F32, tag="xg")
                for m in range(GC):
                    proj_mgroup(w, xh, xl, xg, m)
                for t in range(WIN):
                    ts = w * WIN + t
                    h16_prev = h16
                    h16 = h16p.tile([128, HC, B], F16, tag="h16")
                    gts = [gps.tile([128, 4, B], F32, tag=f"g{c}",
                                    name=f"gt{c}")
                           for c in range(HC)]
                    em_on = ts >= W + 1
                    if em_on:
                        t0 = ts - 1 - W
                        pe_em = emps.tile([B, T], F32, tag="em")
                    # k-outer: (group, k) matmul depends only on h16_prev[k]
                    for k in range(HC):
                        for c in range(HC):
                            for mi in range(4):
                                m = 4 * c + mi
                                nc.tensor.matmul(
                                    gts[c][:, mi, :],
                                    whh_sb[:, k, m * 128:(m + 1) * 128],
                                    h16_prev[:, k, :],
                                    start=(k == 0 and mi == 0),
                                    stop=(k == HC - 1),
                                    skip_group_check=True)
                        if em_on:
                            nc.tensor.matmul(pe_em[:], h16_prev[:, k, :],
                                             weff_sb[:, k, :],
                                             start=(k == 0),
                                             stop=(k == HC - 1),
                                             skip_group_check=True)
                    if em_on:
                        off = t0 + pb * (CH - 1 - 2 * t0)
                        nc.vector.tensor_copy(
                            emT_sb[:, bass.ds(off, 1), :], pe_em[:])
                    for c in range(HC):
                        # gate tiles per chunk: [i, f, o, g]
                        gsb = gtp.tile([128, 4, B], F32, tag="gsb")
                        nc.vector.tensor_add(gsb[:], gts[c][:],
                                             xg[:, 4 * c:4 * c + 4, t, :])
                        sio = gtp.tile([128, 3, B], F32, tag="sio")
                        nc.scalar.activation(sio[:], gsb[:, 0:3, :],
                                             AF.Sigmoid)
                        tg = gtp.tile([128, 1, B], F32, tag="tg")
                        nc.scalar.activation(tg[:], gsb[:, 3:4, :], AF.Tanh)
                        t1 = gtp.tile([128, 1, B], F32, tag="t1")
                        nc.gpsimd.tensor_mul(t1[:, 0, :], sio[:, 1, :],
                                             c32[:, c, :])
                        t2 = gtp.tile([128, 1, B], F32, tag="t2")
                        nc.gpsimd.tensor_mul(t2[:, 0, :], sio[:, 0, :],
                                             tg[:, 0, :])
                        nc.gpsimd.tensor_add(c32[:, c, :], t1[:, 0, :],
                                             t2[:, 0, :])
                        tc_ = gtp.tile([128, 1, B], F32, tag="tc_")
                        nc.scalar.activation(tc_[:], c32[:, c:c + 1, :],
                                             AF.Tanh)
                        nc.vector.tensor_mul(h16[:, c, :], sio[:, 2, :],
                                             tc_[:, 0, :])
            # final step's emission
            pe_em = emps.tile([B, T], F32, tag="em")
            for c in range(HC):
                nc.tensor.matmul(pe_em[:], h16[:, c, :], weff_sb[:, c, :],
                                 start=(c == 0), stop=(c == HC - 1),
                                 skip_group_check=True)
            t0 = CH - 1
            off = t0 + pb * (CH - 1 - 2 * t0)
            nc.vector.tensor_copy(emT_sb[:, bass.ds(off, 1), :], pe_em[:])
            nc.sync.dma_start(emT_bounce.ap()[:], emT_sb[:])

        # ---------- phase C: allgather ----------
        nc.gpsimd.collective_compute(
            "AllGather", ALU.bypass,
            replica_groups=[list(range(8))],
            ins=[emT_bounce.ap()],
            outs=[emg_bounce.ap()],
        )

        # ---------- phase D: viterbi ----------
        with tc.tile_pool(name="vg", bufs=1) as vgp, \
             tc.tile_pool(name="vt", bufs=4) as vtp, \
             tc.tile_pool(name="vh", bufs=1) as vhp:
            emg_sb = vgp.tile([B, 8, CH, T], F16)
            nc.sync.dma_start(emg_sb[:],
                              emg_bounce.ap().rearrange("c b s t -> b c s t"))
            masknot_sb = vgp.tile([B, PADW], U8)
            nc.sync.dma_start(masknot_sb[:], masknot)

            em_full = vgp.tile([B, PADW, T], F32)
            nc.vector.memset(em_full[:, 0:VW, :], 0.0)
            nc.vector.memset(em_full[:, VW + S:, :], 0.0)
            for g in range(4):
                nc.vector.tensor_add(
                    em_full[:, VW + CH * g:VW + CH * (g + 1), :],
                    emg_sb[:, g, :, :],
                    emg_sb[:, 7 - g, :, :])

            vbase = nc.partition_id() * VCH

            hist = vhp.tile([B, SPAN_V, T], F32)
            aspan = vhp.tile([B, SPAN_V, T, T], F32)
            emf_sl = em_full[:, bass.ds(vbase, SPAN_V), :]
            tr_b = bass.AP(tensor=transb_sb.tensor,
                           offset=transb_sb[:].offset,
                           ap=[list(transb_sb[:].ap[0]), [0, SPAN_V], [T, T],
                               [1, T]])
            dg_ap = diagm_sb[:]
            dg_b = bass.AP(tensor=dg_ap.tensor, offset=dg_ap.offset,
                           ap=[list(dg_ap.ap[0]), [0, SPAN_V], [T, T],
                               [1, T]])
            mk_sp = vgp.tile([B, SPAN_V], U8)
            nc.vector.tensor_copy(mk_sp[:],
                                  masknot_sb[:, bass.ds(vbase, SPAN_V)])
            mk_ap2 = mk_sp[:]
            mk_b2 = bass.AP(tensor=mk_ap2.tensor, offset=mk_ap2.offset,
                            ap=[list(mk_ap2.ap[0]), [1, SPAN_V], [0, T],
                                [0, T]])

            # build aspan in halves: gpsimd add, DVE mask-predication
            HALF = SPAN_V // 2
            for lo in (0, HALF):
                hi = lo + HALF
                em_b = bass.AP(tensor=emf_sl.tensor,
                               offset=emf_sl.offset + lo * T,
                               ap=[list(emf_sl.ap[0]), [T, HALF], [1, T],
                                   [0, T]])
                trb_h = bass.AP(tensor=tr_b.tensor, offset=tr_b.offset,
                                ap=[list(tr_b.ap[0]), [0, HALF], [T, T],
                                    [1, T]])
                dgb_h = bass.AP(tensor=dg_b.tensor, offset=dg_b.offset,
                                ap=[list(dg_b.ap[0]), [0, HALF], [T, T],
                                    [1, T]])
                mkb_h = bass.AP(tensor=mk_b2.tensor, offset=mk_b2.offset + lo,
                                ap=[list(mk_b2.ap[0]), [1, HALF], [0, T],
                                    [0, T]])
                nc.vector.tensor_add(aspan[:, lo:hi, :, :], em_b, trb_h)
                nc.vector.copy_predicated(aspan[:, lo:hi, :, :], mkb_h,
                                          dgb_h)

            score = vtp.tile([B, T], F32, tag="vscore")
            t0v = vtp.tile([B, T], F32, tag="vt0")
            nc.vector.tensor_mul(t0v[:], vinitsel_sb[:],
                                 em_full[:, bass.ds(vbase + VW, 1), :])
            nc.vector.tensor_add(score[:], t0v[:], vinit_sb[:])

            for li in range(1, SPAN_V):
                tmp = vtp.tile([B, T, T], F32, tag="vtmp")
                nc.vector.tensor_add(tmp[:], aspan[:, li, :, :],
                                     _bc(score[:], T, 0))
                score = vtp.tile([B, T], F32, tag="vscore")
                nc.vector.reduce_max(score[:], tmp[:],
                                     axis=mybir.AxisListType.X)
                eq = vtp.tile([B, T, T], F32, tag="veq")
                nc.vector.tensor_tensor(eq[:], tmp[:], _bc(score[:], T, 1),
                                        op=ALU.is_equal)
                idm = vtp.tile([B, T, T], F32, tag="vidm")
                nc.gpsimd.tensor_mul(idm[:], eq[:], iota81_sb[:])
                nc.vector.reduce_max(hist[:, li, :], idm[:],
                                     axis=mybir.AxisListType.X)

            nc.vector.tensor_add(score[:], score[:], end9_sb[:])
            mxf = vtp.tile([B, 1], F32, tag="vmxf")
            nc.vector.reduce_max(mxf[:], score[:], axis=mybir.AxisListType.X)
            eqf = vtp.tile([B, T], F32, tag="veqf")
            nc.vector.tensor_tensor(eqf[:], score[:], _bccol(mxf[:], T),
                                    op=ALU.is_equal)
            idf = vtp.tile([B, T], F32, tag="vidf")
            nc.vector.tensor_mul(idf[:], eqf[:], iota9_sb[:])
            tag = vgp.tile([B, 1], F32)
            nc.vector.reduce_max(tag[:], idf[:], axis=mybir.AxisListType.X)

            tags_f = vgp.tile([B, VCH], F32)
            for li in range(SPAN_V - 1, VW - 1, -1):
                if VW <= li < VW + VCH:
                    nc.vector.tensor_copy(tags_f[:, li - VW:li - VW + 1],
                                          tag[:])
                if li == VW:
                    break
                oh = vtp.tile([B, T], F32, tag="voh")
                nc.vector.tensor_tensor(oh[:], iota9_sb[:], _bccol(tag[:], T),
                                        op=ALU.is_equal)
                ohh = vtp.tile([B, T], F32, tag="vohh")
                nc.vector.tensor_mul(ohh[:], oh[:], hist[:, li, :])
                nc.vector.reduce_max(tag[:], ohh[:],
                                     axis=mybir.AxisListType.X)

            tags_i = vgp.tile([B, VCH], I32)
            nc.vector.tensor_copy(tags_i[:], tags_f[:])
            nc.sync.dma_start(tags_out[:], tags_i[:])

    nc.compile()
    return nc


def _host_prep(inputs):
    f32 = np.float32
    bert = np.asarray(inputs["bert_out"], f32)
    mask = np.asarray(inputs["mask"]).astype(bool)

    # gate-tile order per chunk: [i, f, o, g]  (pytorch rows are i,f,g,o)
    qorder = [0, 1, 3, 2]
    perm = np.concatenate([
        np.arange(q * H + c * 128, q * H + (c + 1) * 128)
        for c in range(HC) for q in qorder])

    def split16(x):
        hi = x.astype(np.float16)
        lo = (x.astype(f32) - hi.astype(f32)).astype(np.float16)
        return hi, lo

    dirs = []
    for d, sfx in enumerate(("f", "b")):
        Wih = np.asarray(inputs[f"Wih_{sfx}"], f32)[perm]     # [1536, 768]
        Whh = np.asarray(inputs[f"Whh_{sfx}"], f32)[perm]     # [1536, 384]
        bias = (np.asarray(inputs[f"bih_{sfx}"], f32)
                + np.asarray(inputs[f"bhh_{sfx}"], f32))[perm]
        wihT = np.ascontiguousarray(Wih.T)                    # [768, 1536]
        whhT = np.ascontiguousarray(Whh.T)                    # [384, 1536]
        wh_hi, wh_lo = split16(wihT)
        wih_hi = wh_hi.reshape(DC, 128, 4 * H).transpose(1, 0, 2).copy()
        wih_lo = wh_lo.reshape(DC, 128, 4 * H).transpose(1, 0, 2).copy()
        whh16 = whhT.astype(np.float16).reshape(HC, 128, 4 * H) \
            .transpose(1, 0, 2).copy()
        bias_m = bias.reshape(GC, 128).T.copy()
        dirs.append(dict(wih_hi=wih_hi, wih_lo=wih_lo, whh16=whh16,
                         bias_m=bias_m))

    W1 = np.asarray(inputs["W1"], np.float64)
    W2 = np.asarray(inputs["W2"], np.float64)
    Wc = np.asarray(inputs["Wc"], np.float64)
    b1 = np.asarray(inputs["b1"], np.float64)
    b2 = np.asarray(inputs["b2"], np.float64)
    bc = np.asarray(inputs["bc"], np.float64)
    W_eff = (Wc @ W2 @ W1).astype(f32)                        # [9, 768]
    b_eff = (Wc @ (W2 @ b1 + b2) + bc).astype(f32)

    trans = np.asarray(inputs["trans"], f32)
    start = np.asarray(inputs["start_trans"], f32)
    end = np.asarray(inputs["end_trans"], f32)

    transb = np.ascontiguousarray(trans.T) + b_eff[:, None]   # [j, i] + beff
    diagm = np.where(np.eye(T, dtype=bool), 0.0, -1e9).astype(f32)
    iota81 = np.tile(np.arange(T, dtype=f32), (T, 1))         # [j, i] = i
    iota9 = np.arange(T, dtype=f32)

    masknot = np.ones((B, PADW), np.uint8)
    masknot[:, VW + 1:VW + S] = 1 - mask[:, 1:].astype(np.uint8)

    in_maps = []
    for core in range(8):
        d, ch = core // 4, core % 4
        dd = dirs[d]
        t0 = ch * CH
        lo = t0 - W
        band = np.zeros((B, SPAN, D), f32)
        if d == 0:
            s0 = max(lo, 0)
            band[:, s0 - lo:, :] = bert[:, s0:t0 + CH, :]
        else:
            idx = (S - 1) - np.arange(lo, t0 + CH)
            valid = idx <= S - 1
            band[:, valid, :] = bert[:, idx[valid], :]
        b_hi, b_lo = split16(band)                            # [B, SPAN, D]

        def xt_layout(x16):
            # [B, SPAN, D] -> [128, DC, SPAN, B]
            return np.ascontiguousarray(
                x16.transpose(2, 1, 0).reshape(DC, 128, SPAN, B)
                .transpose(1, 0, 2, 3))

        bias_wv = dd["bias_m"].copy()
        if ch == 0:
            bias_wv[:, [0, 4, 8]] = -30.0   # freeze i-gate during warmup
        weff_half = W_eff[:, d * H:(d + 1) * H].T              # [384, 9]
        weff_t = weff_half.reshape(HC, 128, T).transpose(1, 0, 2) \
            .astype(np.float16).copy()

        end9d = np.broadcast_to(end, (B, T)).copy() if core == 7 \
            else np.zeros((B, T), f32)
        vinitv = np.broadcast_to(start + b_eff, (B, T)).copy() if core == 0 \
            else np.zeros((B, T), f32)
        vinitsel = np.full((B, T), 1.0 if core == 0 else 0.0, f32)

        im = dict(
            xt_hi=xt_layout(b_hi), xt_lo=xt_layout(b_lo),
            wih_hi=dd["wih_hi"], whh16=dd["whh16"],
            bias_w=bias_wv, bias_m=dd["bias_m"],
            weff16=weff_t,
            transb=transb, diagm=diagm, iota81=iota81, iota9=iota9,
            end9d=end9d, vinit=vinitv, vinitsel=vinitsel,
            masknot=masknot,
        )
        if PASSES >= 3:
            im["wih_lo"] = dd["wih_lo"]
        in_maps.append(im)
    return in_maps


def kernel(**inputs):
    global _cache
    if "nc" not in _cache:
        _cache["nc"] = _build()
    nc = _cache["nc"]
    in_maps = _host_prep(inputs)
    res = run_bass_kernel_spmd(nc, in_maps, core_ids=list(range(8)))
    tags = np.concatenate([res.results[c]["tags"] for c in range(8)], axis=1)
    return tags.astype(np.int32)
